# revision 2
# baseline (speedup 1.0000x reference)
"""ConvolutionalAttention (training branch) for Trainium2, 8 NeuronCores.

The module computes, per sample b:
    out[:, :32]  = conv13x13(x1, lk_filter) + depthwise3x3(x1, dyn_k[b])
    out[:, 32:]  = x2            (pass-through)
where dyn_k[b] comes from a tiny MLP (pool -> 1x1 -> GELU -> 1x1) on x1.

Key transformation: conv is linear in the filter, so the per-sample dynamic
depthwise 3x3 kernel is folded host-side into the center of a per-sample
13x13 dense filter.  The device then runs ONE dense 32->32 13x13 conv per
sample.  Data-parallel over batch: 2 samples per core.

Device mapping (per core, per sample):
  - conv as matmul with K = 128 = (4 row-shift replicas g) x (32 in-ch),
    M = 128 = (4 output rows dy) x (32 out-ch), rhs free dim N = 408 =
    two 4-row "quads" (8 output rows) read via an overlapped access pattern.
  - 52 weight blocks (4 ky'-chunks x 13 kx shifts) accumulate in PSUM.
  - float32r matmuls (tf32-like): full PE rate with ~1e-4 relative error.
"""

import json

import numpy as np

import concourse.bass as bass
import concourse.mybir as mybir
import concourse.tile as tile
from concourse.bass_utils import run_bass_kernel_spmd

# ---------------------------------------------------------------------------
# Problem constants (hardcoded; kernel.py must be self-contained)
B, C, H, W = 16, 64, 192, 192
PD, SK, LK = 32, 3, 13
PAD = LK // 2                      # 6
NCORES = 8
BLOC = B // NCORES                 # 2 samples per core
PADW = W + 2 * PAD                 # 204
PADH = H + 2 * PAD                 # 204
NJ, G, DY = 4, 4, 4                # ky' chunks, row-shift replicas, rows/quad
NKX = LK                           # 13 kx shifts
NBLK = NJ * NKX                    # 52 weight blocks per sample
BANDS = 6                          # 32 output rows per band
PAIRS = 4                          # quad-pairs per band (8 rows each)
SROWS = 41                         # X4 rows needed per band
NFREE = 2 * PADW                   # 408 matmul moving free dim
F32 = mybir.dt.float32
F32R = mybir.dt.float32r

# ---------------------------------------------------------------------------
# Workaround: the walrus_driver in this container rejects instructions with
# more than one sync-wait command.  Post-process the BIR JSON, moving excess
# waits onto single-wait NoOps inserted right before the offending
# instruction (same engine => executes first, semantics preserved).
_orig_to_json_bytes = bass.Bass.to_json_bytes


def _split_multi_waits(m):
    for f in m.get("functions", []):
        for blk in f.get("blocks", []):
            out = []
            changed = False
            for inst in blk.get("instructions", []):
                si = inst.get("sync_info")
                waits = (si or {}).get("on_wait") or []
                if len(waits) > 1:
                    changed = True
                    for k, wcond in enumerate(waits[:-1]):
                        out.append({
                            "debug": inst.get("debug"),
                            "engine": inst["engine"],
                            "ins": [], "outs": [],
                            "name": f"{inst['name']}.sw{k}",
                            "opcode": "NoOp",
                            "sync_info": {"on_update": [], "on_wait": [wcond]},
                            "text_hint": "split_wait",
                        })
                    si["on_wait"] = [waits[-1]]
                out.append(inst)
            if changed:
                blk["instructions"] = out
    return m


def _to_json_bytes_split(self, *a, **kw):
    data = _orig_to_json_bytes(self, *a, **kw)
    return json.dumps(_split_multi_waits(json.loads(data))).encode()


def _install_patch():
    if bass.Bass.to_json_bytes is not _to_json_bytes_split:
        bass.Bass.to_json_bytes = _to_json_bytes_split


# ---------------------------------------------------------------------------
# Device kernel


def _build_nc():
    _install_patch()
    nc = bass.Bass()
    xin = nc.declare_dram_parameter("xin", [BLOC, PD, PADH, PADW], F32,
                                    isOutput=False)
    wts = nc.declare_dram_parameter("wts", [BLOC, NJ, NKX, 128, 128], F32,
                                    isOutput=False)
    yout = nc.declare_dram_parameter("yout", [BLOC, PD, H, W], F32,
                                     isOutput=True)
    yout_ap = yout.ap()

    with tile.TileContext(nc) as tc:
        with tc.tile_pool(name="wp", bufs=2) as wp, \
             tc.tile_pool(name="xp", bufs=2) as xp, \
             tc.tile_pool(name="pp", bufs=8, space="PSUM") as pp, \
             tc.tile_pool(name="op", bufs=4) as op:
            for b in range(BLOC):
                wt = wp.tile([128, NBLK * 128], F32R, tag="wt")
                nc.sync.dma_start(
                    wt[:].rearrange("k (j x m) -> k j x m", j=NJ, x=NKX),
                    wts.ap()[b].bitcast(F32R).rearrange("j x k m -> k j x m"))
                for band in range(BANDS):
                    y0 = 32 * band
                    x4 = xp.tile([128, SROWS * PADW + 16], F32R, tag="x4")
                    for g in range(G):
                        nc.sync.dma_start(
                            x4[32 * g:32 * (g + 1), :SROWS * PADW]
                            .rearrange("p (s c) -> p s c", c=PADW),
                            xin.ap()[b, :, y0 + g:y0 + g + SROWS, :]
                            .bitcast(F32R))
                    x4a = x4[:]
                    for p in range(PAIRS):
                        acc = pp.tile([128, NFREE], F32, tag="acc")
                        first = True
                        for j in range(NJ):
                            s0 = 8 * p + 4 * j
                            for kx in range(NKX):
                                rhs = bass.AP(
                                    x4a.tensor,
                                    x4a.offset + s0 * PADW + kx,
                                    [list(x4a.ap[0]),
                                     [4 * PADW, 2], [1, PADW]])
                                nc.tensor.matmul(
                                    acc[:],
                                    wt[:, (j * NKX + kx) * 128:
                                           (j * NKX + kx + 1) * 128],
                                    rhs,
                                    start=first,
                                    stop=(j == NJ - 1 and kx == NKX - 1))
                                first = False
                        ot = op.tile([128, NFREE], F32, tag="ot")
                        nc.vector.tensor_copy(ot[:], acc[:])
                        for q in range(2):
                            src = ot[:, q * PADW:q * PADW + W]
                            dst = bass.AP(
                                yout_ap.tensor,
                                b * PD * H * W + (y0 + 8 * p + 4 * q) * W,
                                [[W, DY], [H * W, PD], [1, W]])
                            nc.sync.dma_start(dst, src)
    return nc


_NC = None


def _get_nc():
    global _NC
    if _NC is None:
        _NC = _build_nc()
    return _NC


# ---------------------------------------------------------------------------
# Host side


def _gelu_exact(z):
    from math import erf
    return 0.5 * z * (1.0 + np.vectorize(erf)(z / np.sqrt(2.0)))


def _prepare_inputs(x, lk_filter, w1, b1, w2, b2):
    x = np.ascontiguousarray(np.asarray(x, dtype=np.float32))
    x1 = x[:, :PD]

    # dwc_proj on host (tiny): pool -> 1x1 -> exact GELU -> 1x1
    pooled = x1.mean(axis=(2, 3), dtype=np.float32)            # [B, 32]
    hid = _gelu_exact(pooled @ np.asarray(w1, np.float32).T
                      + np.asarray(b1, np.float32)).astype(np.float32)
    dyn_k = (hid @ np.asarray(w2, np.float32).T
             + np.asarray(b2, np.float32)).reshape(B, PD, SK, SK)

    # fold the per-sample depthwise 3x3 into the center of the 13x13 filter
    F = np.broadcast_to(np.asarray(lk_filter, np.float32),
                        (B, PD, PD, LK, LK)).copy()
    idx = np.arange(PD)
    ctr = PAD - SK // 2                                         # 5
    F[:, idx, idx, ctr:ctr + SK, ctr:ctr + SK] += dyn_k

    # weight blocks: wts[b, j, kx, g*32+ic, dy*32+oc] = F[b, oc, ic, 4j+g-dy, kx]
    wts = np.zeros((B, NJ, NKX, 128, 128), np.float32)
    for j in range(NJ):
        for g in range(G):
            for dy in range(DY):
                ky = 4 * j + g - dy
                if 0 <= ky < LK:
                    wts[:, j, :, g * PD:(g + 1) * PD,
                        dy * PD:(dy + 1) * PD] = \
                        F[:, :, :, ky, :].transpose(0, 3, 2, 1)

    xpad = np.zeros((B, PD, PADH, PADW), np.float32)
    xpad[:, :, PAD:PAD + H, PAD:PAD + W] = x1

    in_maps = [{"xin": xpad[BLOC * c:BLOC * (c + 1)],
                "wts": wts[BLOC * c:BLOC * (c + 1)]}
               for c in range(NCORES)]
    return x, in_maps


def _execute(in_maps, trace=False):
    nc = _get_nc()
    return run_bass_kernel_spmd(nc, in_maps, list(range(NCORES)), trace=trace)


def kernel(x, lk_filter, w1, b1, w2, b2):
    x, in_maps = _prepare_inputs(x, lk_filter, w1, b1, w2, b2)
    res = _execute(in_maps)
    out = np.empty((B, C, H, W), np.float32)
    for c in range(NCORES):
        out[BLOC * c:BLOC * (c + 1), :PD] = res.results[c]["yout"]
    out[:, PD:] = x[:, PD:]
    return out


# revision 3
# speedup vs baseline: 1.0012x; 1.0012x over previous
"""ConvolutionalAttention (training branch) for Trainium2, 8 NeuronCores.

The module computes, per sample b:
    out[:, :32]  = conv13x13(x1, lk_filter) + depthwise3x3(x1, dyn_k[b])
    out[:, 32:]  = x2            (pass-through)
where dyn_k[b] comes from a tiny MLP (pool -> 1x1 -> GELU -> 1x1) on x1.

Key transformation: conv is linear in the filter, so the per-sample dynamic
depthwise 3x3 kernel is folded host-side into the center of a per-sample
13x13 dense filter.  The device then runs ONE dense 32->32 13x13 conv per
sample.  Data-parallel over batch: 2 samples per core.

Device mapping (per core, per sample):
  - conv as matmul with K = 128 = (4 row-shift replicas g) x (32 in-ch),
    M = 128 = (4 output rows dy) x (32 out-ch), rhs free dim N = 408 =
    two 4-row "quads" (8 output rows) read via an overlapped access pattern.
  - 52 weight blocks (4 ky'-chunks x 13 kx shifts) accumulate in PSUM.
  - float32r matmuls (tf32-like): full PE rate with ~1e-4 relative error.
"""

import json

import numpy as np

import concourse.bass as bass
import concourse.mybir as mybir
import concourse.tile as tile
from concourse.bass_utils import run_bass_kernel_spmd

# ---------------------------------------------------------------------------
# Problem constants (hardcoded; kernel.py must be self-contained)
B, C, H, W = 16, 64, 192, 192
PD, SK, LK = 32, 3, 13
PAD = LK // 2                      # 6
NCORES = 8
BLOC = B // NCORES                 # 2 samples per core
PADW = W + 2 * PAD                 # 204
PADH = H + 2 * PAD                 # 204
NJ, G, DY = 4, 4, 4                # ky' chunks, row-shift replicas, rows/quad
NKX = LK                           # 13 kx shifts
NBLK = NJ * NKX                    # 52 weight blocks per sample
BANDS = 6                          # 32 output rows per band
PAIRS = 4                          # quad-pairs per band (8 rows each)
SROWS = 41                         # X4 rows needed per band
NFREE = 2 * PADW                   # 408 matmul moving free dim
F32 = mybir.dt.float32
F32R = mybir.dt.float32r

# ---------------------------------------------------------------------------
# Workaround: the walrus_driver in this container rejects instructions with
# more than one sync-wait command.  Post-process the BIR JSON, moving excess
# waits onto single-wait NoOps inserted right before the offending
# instruction (same engine => executes first, semantics preserved).
_orig_to_json_bytes = bass.Bass.to_json_bytes


def _split_multi_waits(m):
    for f in m.get("functions", []):
        for blk in f.get("blocks", []):
            out = []
            changed = False
            for inst in blk.get("instructions", []):
                si = inst.get("sync_info")
                waits = (si or {}).get("on_wait") or []
                if len(waits) > 1:
                    changed = True
                    for k, wcond in enumerate(waits[:-1]):
                        out.append({
                            "debug": inst.get("debug"),
                            "engine": inst["engine"],
                            "ins": [], "outs": [],
                            "name": f"{inst['name']}.sw{k}",
                            "opcode": "NoOp",
                            "sync_info": {"on_update": [], "on_wait": [wcond]},
                            "text_hint": "split_wait",
                        })
                    si["on_wait"] = [waits[-1]]
                out.append(inst)
            if changed:
                blk["instructions"] = out
    return m


def _to_json_bytes_split(self, *a, **kw):
    data = _orig_to_json_bytes(self, *a, **kw)
    return json.dumps(_split_multi_waits(json.loads(data))).encode()


def _install_patch():
    if bass.Bass.to_json_bytes is not _to_json_bytes_split:
        bass.Bass.to_json_bytes = _to_json_bytes_split


# ---------------------------------------------------------------------------
# Device kernel


def _build_nc():
    _install_patch()
    nc = bass.Bass()
    xin = nc.declare_dram_parameter("xin", [BLOC, PD, PADH, PADW], F32,
                                    isOutput=False)
    wts = nc.declare_dram_parameter("wts", [BLOC, NJ, NKX, 128, 128], F32,
                                    isOutput=False)
    yout = nc.declare_dram_parameter("yout", [BLOC, PD, H, W], F32,
                                     isOutput=True)
    yout_ap = yout.ap()

    with tile.TileContext(nc) as tc:
        with tc.tile_pool(name="wp", bufs=2) as wp, \
             tc.tile_pool(name="xp", bufs=3) as xp, \
             tc.tile_pool(name="pp", bufs=8, space="PSUM") as pp, \
             tc.tile_pool(name="op", bufs=4) as op:
            for b in range(BLOC):
                wt = wp.tile([128, NBLK * 128], F32R, tag="wt")
                nc.sync.dma_start(
                    wt[:].rearrange("k (j x m) -> k j x m", j=NJ, x=NKX),
                    wts.ap()[b].bitcast(F32R).rearrange("j x k m -> k j x m"))
                for band in range(BANDS):
                    y0 = 32 * band
                    x4 = xp.tile([128, SROWS * PADW + 16], F32R, tag="x4")
                    for g in range(G):
                        nc.sync.dma_start(
                            x4[32 * g:32 * (g + 1), :SROWS * PADW]
                            .rearrange("p (s c) -> p s c", c=PADW),
                            xin.ap()[b, :, y0 + g:y0 + g + SROWS, :]
                            .bitcast(F32R))
                    x4a = x4[:]
                    for p in range(PAIRS):
                        acc = pp.tile([128, NFREE], F32, tag="acc")
                        first = True
                        for j in range(NJ):
                            s0 = 8 * p + 4 * j
                            for kx in range(NKX):
                                rhs = bass.AP(
                                    x4a.tensor,
                                    x4a.offset + s0 * PADW + kx,
                                    [list(x4a.ap[0]),
                                     [4 * PADW, 2], [1, PADW]])
                                nc.tensor.matmul(
                                    acc[:],
                                    wt[:, (j * NKX + kx) * 128:
                                           (j * NKX + kx + 1) * 128],
                                    rhs,
                                    start=first,
                                    stop=(j == NJ - 1 and kx == NKX - 1))
                                first = False
                        ot = op.tile([128, NFREE], F32, tag="ot")
                        nc.vector.tensor_copy(ot[:], acc[:])
                        for q in range(2):
                            src = ot[:, q * PADW:q * PADW + W]
                            dst = bass.AP(
                                yout_ap.tensor,
                                b * PD * H * W + (y0 + 8 * p + 4 * q) * W,
                                [[W, DY], [H * W, PD], [1, W]])
                            nc.sync.dma_start(dst, src)
    return nc


_NC = None


def _get_nc():
    global _NC
    if _NC is None:
        _NC = _build_nc()
    return _NC


# ---------------------------------------------------------------------------
# Host side


def _gelu_exact(z):
    from math import erf
    return 0.5 * z * (1.0 + np.vectorize(erf)(z / np.sqrt(2.0)))


def _prepare_inputs(x, lk_filter, w1, b1, w2, b2):
    x = np.ascontiguousarray(np.asarray(x, dtype=np.float32))
    x1 = x[:, :PD]

    # dwc_proj on host (tiny): pool -> 1x1 -> exact GELU -> 1x1
    pooled = x1.mean(axis=(2, 3), dtype=np.float32)            # [B, 32]
    hid = _gelu_exact(pooled @ np.asarray(w1, np.float32).T
                      + np.asarray(b1, np.float32)).astype(np.float32)
    dyn_k = (hid @ np.asarray(w2, np.float32).T
             + np.asarray(b2, np.float32)).reshape(B, PD, SK, SK)

    # fold the per-sample depthwise 3x3 into the center of the 13x13 filter
    F = np.broadcast_to(np.asarray(lk_filter, np.float32),
                        (B, PD, PD, LK, LK)).copy()
    idx = np.arange(PD)
    ctr = PAD - SK // 2                                         # 5
    F[:, idx, idx, ctr:ctr + SK, ctr:ctr + SK] += dyn_k

    # weight blocks: wts[b, j, kx, g*32+ic, dy*32+oc] = F[b, oc, ic, 4j+g-dy, kx]
    wts = np.zeros((B, NJ, NKX, 128, 128), np.float32)
    for j in range(NJ):
        for g in range(G):
            for dy in range(DY):
                ky = 4 * j + g - dy
                if 0 <= ky < LK:
                    wts[:, j, :, g * PD:(g + 1) * PD,
                        dy * PD:(dy + 1) * PD] = \
                        F[:, :, :, ky, :].transpose(0, 3, 2, 1)

    xpad = np.zeros((B, PD, PADH, PADW), np.float32)
    xpad[:, :, PAD:PAD + H, PAD:PAD + W] = x1

    in_maps = [{"xin": xpad[BLOC * c:BLOC * (c + 1)],
                "wts": wts[BLOC * c:BLOC * (c + 1)]}
               for c in range(NCORES)]
    return x, in_maps


def _execute(in_maps, trace=False):
    nc = _get_nc()
    return run_bass_kernel_spmd(nc, in_maps, list(range(NCORES)), trace=trace)


def kernel(x, lk_filter, w1, b1, w2, b2):
    x, in_maps = _prepare_inputs(x, lk_filter, w1, b1, w2, b2)
    res = _execute(in_maps)
    out = np.empty((B, C, H, W), np.float32)
    for c in range(NCORES):
        out[BLOC * c:BLOC * (c + 1), :PD] = res.results[c]["yout"]
    out[:, PD:] = x[:, PD:]
    return out


# revision 6
# speedup vs baseline: 1.3987x; 1.3970x over previous
"""ConvolutionalAttention (training branch) for Trainium2, 8 NeuronCores.

The module computes, per sample b:
    out[:, :32]  = conv13x13(x1, lk_filter) + depthwise3x3(x1, dyn_k[b])
    out[:, 32:]  = x2            (pass-through)
where dyn_k[b] comes from a tiny MLP (pool -> 1x1 -> GELU -> 1x1) on x1.

Key transformation: conv is linear in the filter, so the per-sample dynamic
depthwise 3x3 kernel is folded host-side into the center of a per-sample
13x13 dense filter.  The device then runs ONE dense 32->32 13x13 conv per
sample.  Data-parallel over batch: 2 samples per core.

Device mapping (per core, per sample):
  - conv as matmul with K = 128 = (4 row-shift replicas g) x (32 in-ch),
    M = 128 = (4 output rows dy) x (32 out-ch), rhs free dim N = 408 =
    two 4-row "quads" (8 output rows) read via an overlapped access pattern.
  - 52 weight blocks (4 ky'-chunks x 13 kx shifts) accumulate in PSUM.
  - float32r matmuls (tf32-like): full PE rate with ~1e-4 relative error.
"""

import json

import numpy as np

import concourse.bass as bass
import concourse.mybir as mybir
import concourse.tile as tile
from concourse.bass_utils import run_bass_kernel_spmd

# ---------------------------------------------------------------------------
# Problem constants (hardcoded; kernel.py must be self-contained)
B, C, H, W = 16, 64, 192, 192
PD, SK, LK = 32, 3, 13
PAD = LK // 2                      # 6
NCORES = 8
BLOC = B // NCORES                 # 2 samples per core
PADW = W + 2 * PAD                 # 204
PADH = H + 2 * PAD                 # 204
NJ, G, DY = 4, 4, 4                # ky' chunks, row-shift replicas, rows/quad
NKX = LK                           # 13 kx shifts
NBLK = NJ * NKX                    # 52 weight blocks per sample
BANDS = 6                          # 32 output rows per band
PAIRS = 4                          # quad-pairs per band (8 rows each)
SROWS = 41                         # X4 rows needed per band
NFREE = 2 * PADW                   # 408 matmul moving free dim
F32 = mybir.dt.float32
F32R = mybir.dt.float32r

# ---------------------------------------------------------------------------
# Workaround: the walrus_driver in this container rejects instructions with
# more than one sync-wait command.  Post-process the BIR JSON, moving excess
# waits onto single-wait NoOps inserted right before the offending
# instruction (same engine => executes first, semantics preserved).
_orig_to_json_bytes = bass.Bass.to_json_bytes


def _split_multi_waits(m):
    for f in m.get("functions", []):
        for blk in f.get("blocks", []):
            out = []
            changed = False
            for inst in blk.get("instructions", []):
                si = inst.get("sync_info")
                waits = (si or {}).get("on_wait") or []
                if len(waits) > 1:
                    changed = True
                    for k, wcond in enumerate(waits[:-1]):
                        out.append({
                            "debug": inst.get("debug"),
                            "engine": inst["engine"],
                            "ins": [], "outs": [],
                            "name": f"{inst['name']}.sw{k}",
                            "opcode": "NoOp",
                            "sync_info": {"on_update": [], "on_wait": [wcond]},
                            "text_hint": "split_wait",
                        })
                    si["on_wait"] = [waits[-1]]
                out.append(inst)
            if changed:
                blk["instructions"] = out
    return m


def _to_json_bytes_split(self, *a, **kw):
    data = _orig_to_json_bytes(self, *a, **kw)
    return json.dumps(_split_multi_waits(json.loads(data))).encode()


def _install_patch():
    if bass.Bass.to_json_bytes is not _to_json_bytes_split:
        bass.Bass.to_json_bytes = _to_json_bytes_split


# ---------------------------------------------------------------------------
# Device kernel


def _build_nc():
    _install_patch()
    nc = bass.Bass()
    xin = nc.declare_dram_parameter("xin", [BLOC, PD, PADH, PADW], F32,
                                    isOutput=False)
    wts = nc.declare_dram_parameter("wts", [BLOC, NJ, NKX, 128, 128], F32,
                                    isOutput=False)
    yout = nc.declare_dram_parameter("yout", [BLOC, PD, H, W], F32,
                                     isOutput=True)
    yout_ap = yout.ap()

    with tile.TileContext(nc) as tc:
        with tc.tile_pool(name="wp", bufs=2) as wp, \
             tc.tile_pool(name="xp", bufs=3) as xp, \
             tc.tile_pool(name="pp", bufs=8, space="PSUM") as pp, \
             tc.tile_pool(name="op", bufs=4) as op:

            def load_band(b, band):
                # input DMAs stay on the sync queue (no output traffic
                # there), so prefetch can run ahead of compute
                y0 = 32 * band
                x4 = xp.tile([128, SROWS * PADW + 16], F32R, tag="x4")
                for g in range(G):
                    nc.sync.dma_start(
                        x4[32 * g:32 * (g + 1), :SROWS * PADW]
                        .rearrange("p (s c) -> p s c", c=PADW),
                        xin.ap()[b, :, y0 + g:y0 + g + SROWS, :]
                        .bitcast(F32R))
                return x4

            def load_wt(b):
                wt = wp.tile([128, NBLK * 128], F32R, tag="wt")
                nc.sync.dma_start(
                    wt[:].rearrange("k (j x m) -> k j x m", j=NJ, x=NKX),
                    wts.ap()[b].bitcast(F32R).rearrange("j x k m -> k j x m"))
                return wt

            steps = [(b, band) for b in range(BLOC) for band in range(BANDS)]
            wtiles = [load_wt(0)]
            x4_next = load_band(*steps[0])
            wtiles.append(load_wt(1))
            for si, (b, band) in enumerate(steps):
                wt = wtiles[b]
                y0 = 32 * band
                x4 = x4_next
                if si + 1 < len(steps):
                    x4_next = load_band(*steps[si + 1])
                x4a = x4[:]
                for p in range(PAIRS):
                    acc = pp.tile([128, NFREE], F32, tag="acc")
                    first = True
                    for j in range(NJ):
                        s0 = 8 * p + 4 * j
                        for kx in range(NKX):
                            rhs = bass.AP(
                                x4a.tensor,
                                x4a.offset + s0 * PADW + kx,
                                [list(x4a.ap[0]),
                                 [4 * PADW, 2], [1, PADW]])
                            nc.tensor.matmul(
                                acc[:],
                                wt[:, (j * NKX + kx) * 128:
                                       (j * NKX + kx + 1) * 128],
                                rhs,
                                start=first,
                                stop=(j == NJ - 1 and kx == NKX - 1))
                            first = False
                    ot = op.tile([128, NFREE], F32, tag="ot")
                    nc.vector.tensor_copy(ot[:], acc[:])
                    for q in range(2):
                        src = ot[:, q * PADW:q * PADW + W]
                        dst = bass.AP(
                            yout_ap.tensor,
                            b * PD * H * W + (y0 + 8 * p + 4 * q) * W,
                            [[W, DY], [H * W, PD], [1, W]])
                        nc.gpsimd.dma_start(dst, src)
    return nc


_NC = None


def _get_nc():
    global _NC
    if _NC is None:
        _NC = _build_nc()
    return _NC


# ---------------------------------------------------------------------------
# Host side


def _gelu_exact(z):
    from math import erf
    return 0.5 * z * (1.0 + np.vectorize(erf)(z / np.sqrt(2.0)))


def _prepare_inputs(x, lk_filter, w1, b1, w2, b2):
    x = np.ascontiguousarray(np.asarray(x, dtype=np.float32))
    x1 = x[:, :PD]

    # dwc_proj on host (tiny): pool -> 1x1 -> exact GELU -> 1x1
    pooled = x1.mean(axis=(2, 3), dtype=np.float32)            # [B, 32]
    hid = _gelu_exact(pooled @ np.asarray(w1, np.float32).T
                      + np.asarray(b1, np.float32)).astype(np.float32)
    dyn_k = (hid @ np.asarray(w2, np.float32).T
             + np.asarray(b2, np.float32)).reshape(B, PD, SK, SK)

    # fold the per-sample depthwise 3x3 into the center of the 13x13 filter
    F = np.broadcast_to(np.asarray(lk_filter, np.float32),
                        (B, PD, PD, LK, LK)).copy()
    idx = np.arange(PD)
    ctr = PAD - SK // 2                                         # 5
    F[:, idx, idx, ctr:ctr + SK, ctr:ctr + SK] += dyn_k

    # weight blocks: wts[b, j, kx, g*32+ic, dy*32+oc] = F[b, oc, ic, 4j+g-dy, kx]
    wts = np.zeros((B, NJ, NKX, 128, 128), np.float32)
    for j in range(NJ):
        for g in range(G):
            for dy in range(DY):
                ky = 4 * j + g - dy
                if 0 <= ky < LK:
                    wts[:, j, :, g * PD:(g + 1) * PD,
                        dy * PD:(dy + 1) * PD] = \
                        F[:, :, :, ky, :].transpose(0, 3, 2, 1)

    xpad = np.zeros((B, PD, PADH, PADW), np.float32)
    xpad[:, :, PAD:PAD + H, PAD:PAD + W] = x1

    in_maps = [{"xin": xpad[BLOC * c:BLOC * (c + 1)],
                "wts": wts[BLOC * c:BLOC * (c + 1)]}
               for c in range(NCORES)]
    return x, in_maps


def _execute(in_maps, trace=False):
    nc = _get_nc()
    return run_bass_kernel_spmd(nc, in_maps, list(range(NCORES)), trace=trace)


def kernel(x, lk_filter, w1, b1, w2, b2):
    x, in_maps = _prepare_inputs(x, lk_filter, w1, b1, w2, b2)
    res = _execute(in_maps)
    out = np.empty((B, C, H, W), np.float32)
    for c in range(NCORES):
        out[BLOC * c:BLOC * (c + 1), :PD] = res.results[c]["yout"]
    out[:, PD:] = x[:, PD:]
    return out


# revision 10
# speedup vs baseline: 1.4091x; 1.0075x over previous
"""ConvolutionalAttention (training branch) for Trainium2, 8 NeuronCores.

The module computes, per sample b:
    out[:, :32]  = conv13x13(x1, lk_filter) + depthwise3x3(x1, dyn_k[b])
    out[:, 32:]  = x2            (pass-through)
where dyn_k[b] comes from a tiny MLP (pool -> 1x1 -> GELU -> 1x1) on x1.

Key transformation: conv is linear in the filter, so the per-sample dynamic
depthwise 3x3 kernel is folded host-side into the center of a per-sample
13x13 dense filter.  The device then runs ONE dense 32->32 13x13 conv per
sample.  Data-parallel over batch: 2 samples per core.

Device mapping (per core, per sample):
  - conv as matmul with K = 128 = (4 row-shift replicas g) x (32 in-ch),
    M = 128 = (4 output rows dy) x (32 out-ch), rhs free dim N = 408 =
    two 4-row "quads" (8 output rows) read via an overlapped access pattern.
  - 52 weight blocks (4 ky'-chunks x 13 kx shifts) accumulate in PSUM.
  - float32r matmuls (tf32-like): full PE rate with ~1e-4 relative error.
"""

import json

import numpy as np

import concourse.bass as bass
import concourse.mybir as mybir
import concourse.tile as tile
from concourse.bass_utils import run_bass_kernel_spmd

# ---------------------------------------------------------------------------
# Problem constants (hardcoded; kernel.py must be self-contained)
B, C, H, W = 16, 64, 192, 192
PD, SK, LK = 32, 3, 13
PAD = LK // 2                      # 6
NCORES = 8
BLOC = B // NCORES                 # 2 samples per core
PADW = W + 2 * PAD                 # 204
PADH = H + 2 * PAD                 # 204
NJ, G, DY = 4, 4, 4                # ky' chunks, row-shift replicas, rows/quad
NKX = LK                           # 13 kx shifts
NBLK = NJ * NKX                    # 52 weight blocks per sample
BANDS = 6                          # 32 output rows per band
PAIRS = 4                          # quad-pairs per band (8 rows each)
SROWS = 41                         # X4 rows needed per band
NFREE = 2 * PADW                   # 408 matmul moving free dim
F32 = mybir.dt.float32
F32R = mybir.dt.float32r

# ---------------------------------------------------------------------------
# Workaround: the walrus_driver in this container rejects instructions with
# more than one sync-wait command.  Post-process the BIR JSON, moving excess
# waits onto single-wait NoOps inserted right before the offending
# instruction (same engine => executes first, semantics preserved).
_orig_to_json_bytes = bass.Bass.to_json_bytes


def _split_multi_waits(m):
    for f in m.get("functions", []):
        for blk in f.get("blocks", []):
            out = []
            changed = False
            for inst in blk.get("instructions", []):
                si = inst.get("sync_info")
                waits = (si or {}).get("on_wait") or []
                if len(waits) > 1:
                    changed = True
                    for k, wcond in enumerate(waits[:-1]):
                        out.append({
                            "debug": inst.get("debug"),
                            "engine": inst["engine"],
                            "ins": [], "outs": [],
                            "name": f"{inst['name']}.sw{k}",
                            "opcode": "NoOp",
                            "sync_info": {"on_update": [], "on_wait": [wcond]},
                            "text_hint": "split_wait",
                        })
                    si["on_wait"] = [waits[-1]]
                out.append(inst)
            if changed:
                blk["instructions"] = out
    return m


def _to_json_bytes_split(self, *a, **kw):
    data = _orig_to_json_bytes(self, *a, **kw)
    return json.dumps(_split_multi_waits(json.loads(data))).encode()


def _install_patch():
    if bass.Bass.to_json_bytes is not _to_json_bytes_split:
        bass.Bass.to_json_bytes = _to_json_bytes_split
    # enable walrus's redundant-LDWEIGHTS elision (off by default); our
    # matmul stream reuses each weight block for 4 consecutive matmuls
    import concourse.bass_utils as _bu
    if not getattr(_bu, "_ldw_opt_patched", False):
        _orig_run_command = _bu.run_command

        def _run_command_ldw(cmd, *a, **kw):
            cmd = ["--enable-ldw-opt=true" if c == "--enable-ldw-opt=false"
                   else c for c in cmd]
            return _orig_run_command(cmd, *a, **kw)

        _bu.run_command = _run_command_ldw
        _bu._ldw_opt_patched = True


# ---------------------------------------------------------------------------
# Device kernel


def _build_nc():
    _install_patch()
    nc = bass.Bass()
    xin = nc.declare_dram_parameter("xin", [BLOC, PD, PADH, PADW], F32,
                                    isOutput=False)
    wts = nc.declare_dram_parameter("wts", [BLOC, NJ, NKX, 128, 128], F32,
                                    isOutput=False)
    yout = nc.declare_dram_parameter("yout", [BLOC, PD, H, W], F32,
                                     isOutput=True)
    yout_ap = yout.ap()

    with tile.TileContext(nc) as tc:
        with tc.tile_pool(name="wp", bufs=2) as wp, \
             tc.tile_pool(name="xp", bufs=3) as xp, \
             tc.tile_pool(name="pp", bufs=2, space="PSUM") as pp, \
             tc.tile_pool(name="op", bufs=4) as op:

            def load_band(b, band):
                # input DMAs stay on the sync queue (no output traffic
                # there), so prefetch can run ahead of compute
                y0 = 32 * band
                x4 = xp.tile([128, SROWS * PADW + 16], F32R, tag="x4")
                for g in range(G):
                    nc.sync.dma_start(
                        x4[32 * g:32 * (g + 1), :SROWS * PADW]
                        .rearrange("p (s c) -> p s c", c=PADW),
                        xin.ap()[b, :, y0 + g:y0 + g + SROWS, :]
                        .bitcast(F32R))
                return x4

            def load_wt(b):
                wt = wp.tile([128, NBLK * 128], F32R, tag="wt")
                nc.sync.dma_start(
                    wt[:].rearrange("k (j x m) -> k j x m", j=NJ, x=NKX),
                    wts.ap()[b].bitcast(F32R).rearrange("j x k m -> k j x m"))
                return wt

            steps = [(b, band) for b in range(BLOC) for band in range(BANDS)]
            wtiles = [load_wt(0)]
            x4_next = load_band(*steps[0])
            wtiles.append(load_wt(1))
            for si, (b, band) in enumerate(steps):
                wt = wtiles[b]
                y0 = 32 * band
                x4 = x4_next
                if si + 1 < len(steps):
                    x4_next = load_band(*steps[si + 1])
                x4a = x4[:]
                # weight-block-outer order: each block feeds all 4 pairs
                # back-to-back so walrus's redundant-LDWEIGHTS elision
                # (--enable-ldw-opt) drops 3 of every 4 weight loads
                accs = [pp.tile([128, NFREE], F32, tag=f"acc{p}",
                                name=f"acc{p}_{si}")
                        for p in range(PAIRS)]
                for j in range(NJ):
                    for kx in range(NKX):
                        wblk = wt[:, (j * NKX + kx) * 128:
                                     (j * NKX + kx + 1) * 128]
                        for p in range(PAIRS):
                            s0 = 8 * p + 4 * j
                            rhs = bass.AP(
                                x4a.tensor,
                                x4a.offset + s0 * PADW + kx,
                                [list(x4a.ap[0]),
                                 [4 * PADW, 2], [1, PADW]])
                            nc.tensor.matmul(
                                accs[p][:], wblk, rhs,
                                start=(j == 0 and kx == 0),
                                stop=(j == NJ - 1 and kx == NKX - 1))
                for p in range(PAIRS):
                    ot = op.tile([128, NFREE], F32, tag="ot")
                    nc.vector.tensor_copy(ot[:], accs[p][:])
                    for q in range(2):
                        src = ot[:, q * PADW:q * PADW + W]
                        dst = bass.AP(
                            yout_ap.tensor,
                            b * PD * H * W + (y0 + 8 * p + 4 * q) * W,
                            [[W, DY], [H * W, PD], [1, W]])
                        nc.gpsimd.dma_start(dst, src)
    return nc


_NC = None


def _get_nc():
    global _NC
    if _NC is None:
        _NC = _build_nc()
    return _NC


# ---------------------------------------------------------------------------
# Host side


def _gelu_exact(z):
    from math import erf
    return 0.5 * z * (1.0 + np.vectorize(erf)(z / np.sqrt(2.0)))


def _prepare_inputs(x, lk_filter, w1, b1, w2, b2):
    x = np.ascontiguousarray(np.asarray(x, dtype=np.float32))
    x1 = x[:, :PD]

    # dwc_proj on host (tiny): pool -> 1x1 -> exact GELU -> 1x1
    pooled = x1.mean(axis=(2, 3), dtype=np.float32)            # [B, 32]
    hid = _gelu_exact(pooled @ np.asarray(w1, np.float32).T
                      + np.asarray(b1, np.float32)).astype(np.float32)
    dyn_k = (hid @ np.asarray(w2, np.float32).T
             + np.asarray(b2, np.float32)).reshape(B, PD, SK, SK)

    # fold the per-sample depthwise 3x3 into the center of the 13x13 filter
    F = np.broadcast_to(np.asarray(lk_filter, np.float32),
                        (B, PD, PD, LK, LK)).copy()
    idx = np.arange(PD)
    ctr = PAD - SK // 2                                         # 5
    F[:, idx, idx, ctr:ctr + SK, ctr:ctr + SK] += dyn_k

    # weight blocks: wts[b, j, kx, g*32+ic, dy*32+oc] = F[b, oc, ic, 4j+g-dy, kx]
    wts = np.zeros((B, NJ, NKX, 128, 128), np.float32)
    for j in range(NJ):
        for g in range(G):
            for dy in range(DY):
                ky = 4 * j + g - dy
                if 0 <= ky < LK:
                    wts[:, j, :, g * PD:(g + 1) * PD,
                        dy * PD:(dy + 1) * PD] = \
                        F[:, :, :, ky, :].transpose(0, 3, 2, 1)

    xpad = np.zeros((B, PD, PADH, PADW), np.float32)
    xpad[:, :, PAD:PAD + H, PAD:PAD + W] = x1

    in_maps = [{"xin": xpad[BLOC * c:BLOC * (c + 1)],
                "wts": wts[BLOC * c:BLOC * (c + 1)]}
               for c in range(NCORES)]
    return x, in_maps


def _execute(in_maps, trace=False):
    nc = _get_nc()
    return run_bass_kernel_spmd(nc, in_maps, list(range(NCORES)), trace=trace)


def kernel(x, lk_filter, w1, b1, w2, b2):
    x, in_maps = _prepare_inputs(x, lk_filter, w1, b1, w2, b2)
    res = _execute(in_maps)
    out = np.empty((B, C, H, W), np.float32)
    for c in range(NCORES):
        out[BLOC * c:BLOC * (c + 1), :PD] = res.results[c]["yout"]
    out[:, PD:] = x[:, PD:]
    return out
